# revision 17
# baseline (speedup 1.0000x reference)
"""Trainium2 Bass kernel for Advanced RGCN (2-layer RGCNConv + LayerNorm + edge-MLP decoder).

Strategy (8 NeuronCores, one SPMD program):
  - Shard by DESTINATION node: core c owns 6250 nodes, padded to 6272 = 49
    windows of 128. Every edge is processed on the core owning its dst, so
    per-relation segment sums are complete locally (no all-reduce of
    aggregates; only a small bf16 AllGather of node features per layer).
  - Aggregation as one-hot matmul: edges are tiled 128 at a time; gathered
    source rows Xg [128e, d] (custom indirect DMA gather) are reduced as
    aggT += Xg^T @ S where S[e, j] = w_e * (dst_local_e == j) is built
    on-chip by one fused tensor_scalar op; w_e = 1/cnt (mean aggregation)
    is precomputed on the host. The root/self term rides along as an extra
    relation whose transform weight is `root`.
  - Transform stays transposed: outT[feat, n] = sum_k Wcat[k]^T @ meanT[k];
    bias+ReLU on ACT; PE-transpose back to [n, feat]; LayerNorm on DVE;
    bf16 cast; DMA to the core's slice; AllGather -> next layer's table.
  - Decoder: dec edges sharded round-robin; transposed gathers produce
    zT [512, e] directly; 3 matmul stages with exact GELU on ACT.
  - dma_gather uses int16 indices, so feature tables are viewed as lo/hi
    halves (25088 rows each); edges are grouped by (window, src-half,
    relation) with cross-core uniform padded tile counts so a single
    program serves all 8 cores.
"""

import os
import sys

sys.path.insert(0, "/opt/trn_rl_repo")

import numpy as np
from ml_dtypes import bfloat16

# ---------------------------------------------------------------- config


class CFG:
    R = 8
    DIN = 128
    H = 256
    NC = 8          # cores
    WIN = 128       # nodes per psum window
    MAXB = 16       # max tiles per gather batch
    DECB = 2048     # dec idxs per gather op
    LN_EPS = 1e-5
    TGROUP = 4      # windows per transform group

    def __init__(self, n_nodes=50000):
        self.N = n_nodes
        self.NPC = self.N // self.NC                     # real nodes per core
        self.NW = (self.NPC + self.WIN - 1) // self.WIN  # windows per core
        self.NPAD = self.NW * self.WIN                   # padded nodes per core
        self.TBL = self.NPAD * self.NC                   # feature-table rows
        assert self.TBL % 2 == 0
        self.HALF = self.TBL // 2
        assert self.HALF <= 32768, "int16 gather index limit"
        assert self.HALF % self.NPAD == 0, "core slices must not straddle lo/hi split"


def tblrow(g, cfg):
    return (g // cfg.NPC) * cfg.NPAD + (g % cfg.NPC)


class Meta:
    pass


# ---------------------------------------------------------------- host preprocessing


def preprocess(x, edge_index, edge_type, dec_edges,
               W1, root1, b1, W2, root2, b2,
               ln1_g, ln1_b, ln2_g, ln2_b,
               mlp_w1, mlp_b1, mlp_w2, mlp_b2, mlp_w3, mlp_b3,
               cfg: CFG):
    m = Meta()
    C, WIN, R = cfg.NC, cfg.WIN, cfg.R
    src_g = np.asarray(edge_index[0], np.int64)
    dst_g = np.asarray(edge_index[1], np.int64)
    et = np.asarray(edge_type, np.int64)

    cnt = np.zeros((R, cfg.N), np.float32)
    np.add.at(cnt, (et, dst_g), 1.0)
    wgt_e = (1.0 / np.maximum(cnt[et, dst_g], 1.0)).astype(np.float32)

    src_t = (src_g // cfg.NPC) * cfg.NPAD + (src_g % cfg.NPC)

    # append self edges (relation index R, weight 1, W[R] = root)
    sg = np.arange(cfg.N, dtype=np.int64)
    s_srct = (sg // cfg.NPC) * cfg.NPAD + (sg % cfg.NPC)
    all_core = np.concatenate([dst_g // cfg.NPC, sg // cfg.NPC])
    all_loc = np.concatenate([dst_g % cfg.NPC, sg % cfg.NPC])
    all_rel = np.concatenate([et, np.full(cfg.N, R, np.int64)])
    all_src = np.concatenate([src_t, s_srct])
    all_w = np.concatenate([wgt_e, np.ones(cfg.N, np.float32)])
    all_win = all_loc // WIN
    all_dl = all_loc % WIN
    all_sec = (all_src >= cfg.HALF).astype(np.int64)

    NR = R + 1
    counts = np.zeros((C, cfg.NW, 2, NR), np.int64)
    np.add.at(counts, (all_core, all_win, all_sec, all_rel), 1)
    T = np.ceil(counts.max(axis=0) / WIN).astype(np.int64)  # [NW, 2, NR]
    # guarantee at least one tile per (w, r) so psum regions are always written
    for w in range(cfg.NW):
        for r in range(NR):
            if T[w, 0, r] + T[w, 1, r] == 0:
                T[w, 0, r] = 1
    m.T = T
    ntiles = int(T.sum())
    m.ntiles = ntiles

    idx_i16 = np.zeros((C, ntiles * WIN), np.int16)
    dl_arr = np.full((C, ntiles * WIN), -1.0, np.float32)
    w_arr = np.zeros((C, ntiles * WIN), np.float32)

    tile_win = np.zeros(ntiles, np.int64)
    tile_sec = np.zeros(ntiles, np.int64)
    tile_rel = np.zeros(ntiles, np.int64)
    slot_of = {}
    t = 0
    for w in range(cfg.NW):
        for s in range(2):
            for r in range(NR):
                nt = int(T[w, s, r])
                slot_of[(w, s, r)] = (t, nt)
                tile_win[t:t + nt] = w
                tile_sec[t:t + nt] = s
                tile_rel[t:t + nt] = r
                t += nt
    assert t == ntiles
    m.tile_win, m.tile_sec, m.tile_rel = tile_win, tile_sec, tile_rel

    order = np.lexsort((all_rel, all_sec, all_win, all_core))
    oc, ow, os_, orl = all_core[order], all_win[order], all_sec[order], all_rel[order]
    osrc, odl, owt = all_src[order], all_dl[order], all_w[order]
    key = ((oc * cfg.NW + ow) * 2 + os_) * NR + orl
    newgrp = np.ones(len(key), bool)
    newgrp[1:] = key[1:] != key[:-1]
    gsp = np.flatnonzero(newgrp)
    within = np.arange(len(key)) - np.repeat(gsp, np.diff(np.append(gsp, len(key))))
    base = np.array([slot_of[(w, s, r)][0] for w, s, r in zip(ow, os_, orl)],
                    np.int64) * WIN
    pos = base + within
    idx_i16[oc, pos] = (osrc - os_ * cfg.HALF).astype(np.int16)
    dl_arr[oc, pos] = odl.astype(np.float32)
    w_arr[oc, pos] = owt

    # gather ops: contiguous tile runs of the same section within a window
    gather_ops = []
    for w in range(cfg.NW):
        for s in range(2):
            t0 = slot_of[(w, s, 0)][0]
            tend = slot_of[(w, s, NR - 1)][0] + slot_of[(w, s, NR - 1)][1]
            run = tend - t0
            o = t0
            while run > 0:
                nt = min(run, cfg.MAXB)
                gather_ops.append((o, nt, s))
                o += nt
                run -= nt
    m.gather_ops = gather_ops

    # idx wrapped [16, ntiles*8]: idx i of tile t -> [i%16, t*8 + i//16]
    tmp = idx_i16.reshape(C, ntiles, 8, 16)
    w16 = np.ascontiguousarray(tmp.transpose(0, 3, 1, 2).reshape(C, 16, ntiles * 8))
    m.idx_wrapped = np.ascontiguousarray(np.tile(w16, (1, 8, 1)))  # [C, 128, T*8]
    m.dl_bf = np.ascontiguousarray(
        dl_arr.reshape(C, ntiles, WIN).transpose(0, 2, 1))
    m.w_bf = np.ascontiguousarray(
        w_arr.reshape(C, ntiles, WIN).transpose(0, 2, 1))

    # feature table for layer 1
    x_tbl = np.zeros((cfg.TBL, cfg.DIN), np.float32)
    x_tbl[tblrow(np.arange(cfg.N), cfg)] = np.asarray(x)
    m.x_tbl_bf = x_tbl.astype(bfloat16)

    # weights
    H = cfg.H
    m.K1 = NR
    Wcat1 = np.zeros((m.K1, cfg.DIN, H), np.float32)
    for r in range(R):
        Wcat1[r] = np.asarray(W1[r])
    Wcat1[R] = np.asarray(root1)
    m.Wcat1_bf = Wcat1.astype(bfloat16)

    m.K2 = 2 * R + 2
    Wcat2 = np.zeros((m.K2, 128, H), np.float32)
    for r in range(R):
        Wcat2[2 * r] = np.asarray(W2[r][:128])
        Wcat2[2 * r + 1] = np.asarray(W2[r][128:])
    Wcat2[2 * R] = np.asarray(root2[:128])
    Wcat2[2 * R + 1] = np.asarray(root2[128:])
    m.Wcat2_bf = Wcat2.astype(bfloat16)

    m.b1_half = np.ascontiguousarray(np.asarray(b1, np.float32).reshape(2, 128).T)
    m.b2_half = np.ascontiguousarray(np.asarray(b2, np.float32).reshape(2, 128).T)
    m.g1_bc = np.broadcast_to(np.asarray(ln1_g, np.float32), (128, H)).copy()
    m.b1ln_bc = np.broadcast_to(np.asarray(ln1_b, np.float32), (128, H)).copy()
    m.g2_bc = np.broadcast_to(np.asarray(ln2_g, np.float32), (128, H)).copy()
    m.b2ln_bc = np.broadcast_to(np.asarray(ln2_b, np.float32), (128, H)).copy()

    m.mlp_w1_bf = np.asarray(mlp_w1).astype(bfloat16)
    m.mlp_w2_bf = np.asarray(mlp_w2).astype(bfloat16)
    m.mlp_w3_bf = np.asarray(mlp_w3).astype(bfloat16)
    m.mlp_b1_half = np.ascontiguousarray(np.asarray(mlp_b1, np.float32).reshape(2, 128).T)
    m.mlp_b2 = np.asarray(mlp_b2, np.float32).reshape(128, 1)
    m.mlp_b3 = np.asarray(mlp_b3, np.float32).reshape(2, 1)

    m.iota_bf = np.broadcast_to(np.arange(128, dtype=np.float32),
                                (128, 128)).astype(bfloat16).copy()
    m.ident_f32 = np.eye(128, dtype=np.float32)

    # decoder edges
    M_ = dec_edges.shape[0]
    d0 = np.asarray(dec_edges[:, 0], np.int64)
    d1 = np.asarray(dec_edges[:, 1], np.int64)
    r0 = tblrow(d0, cfg)
    r1 = tblrow(d1, cfg)
    dcore = np.arange(M_) % C
    bucket = (r0 >= cfg.HALF) * 2 + (r1 >= cfg.HALF)
    bc = np.zeros((C, 4), np.int64)
    np.add.at(bc, (dcore, bucket), 1)
    bpad = ((bc.max(axis=0) + 127) // 128) * 128
    m.dec_bpad = bpad
    DEC_TOT = int(bpad.sum())
    m.DEC_TOT = DEC_TOT

    dec_idx0 = np.zeros((C, DEC_TOT), np.int16)
    dec_idx1 = np.zeros((C, DEC_TOT), np.int16)
    dec_map = np.full((C, DEC_TOT), -1, np.int64)
    boff = np.concatenate([[0], np.cumsum(bpad)])[:4].astype(np.int64)
    order2 = np.lexsort((bucket, dcore))
    oc2, ob2 = dcore[order2], bucket[order2]
    key2 = oc2 * 4 + ob2
    newg = np.ones(len(key2), bool)
    newg[1:] = key2[1:] != key2[:-1]
    gsp2 = np.flatnonzero(newg)
    within2 = np.arange(len(key2)) - np.repeat(gsp2, np.diff(np.append(gsp2, len(key2))))
    pos2 = boff[ob2] + within2
    dec_idx0[oc2, pos2] = (r0[order2] - (ob2 // 2) * cfg.HALF).astype(np.int16)
    dec_idx1[oc2, pos2] = (r1[order2] - (ob2 % 2) * cfg.HALF).astype(np.int16)
    dec_map[oc2, pos2] = order2
    m.dec_map = dec_map

    dec_ops = []
    for b in range(4):
        off = int(boff[b])
        rem = int(bpad[b])
        while rem > 0:
            n = min(rem, cfg.DECB)
            dec_ops.append((off, n, b // 2, b % 2))
            off += n
            rem -= n
    m.dec_ops = dec_ops

    def wrap16(a):
        Cc, L = a.shape
        w = np.ascontiguousarray(a.reshape(Cc, L // 16, 16).transpose(0, 2, 1))
        return np.ascontiguousarray(np.tile(w, (1, 8, 1)))  # [C, 128, L/16]

    m.dec_idx0_w = wrap16(dec_idx0)
    m.dec_idx1_w = wrap16(dec_idx1)
    m.n_dec = M_
    return m


# ---------------------------------------------------------------- numpy emulator


def emulate(m: Meta, cfg: CFG, mlp_b1, mlp_b2, mlp_b3, sim_gelu=False):
    """Emulates the planned device computation in fp32 numpy to validate the
    host-built streams/layout (not bf16-exact)."""
    from scipy.special import erf

    C, WIN = cfg.NC, cfg.WIN
    f32 = np.float32
    tbl = np.array(m.x_tbl_bf, f32)
    Wcats = [np.array(m.Wcat1_bf, f32), np.array(m.Wcat2_bf, f32)]
    biases = [m.b1_half, m.b2_half]
    gs = [m.g1_bc[0], m.g2_bc[0]]
    bs = [m.b1ln_bc[0], m.b2ln_bc[0]]

    def layer(li, feats, d):
        K = [m.K1, m.K2][li]
        out_all = np.zeros((cfg.TBL, cfg.H), f32)
        for c in range(C):
            dl = np.array(m.dl_bf[c], f32)
            wv = np.array(m.w_bf[c], f32)
            idxw = m.idx_wrapped[c]
            meanT = np.zeros((K, 128, cfg.NPAD), f32)
            for t in range(m.ntiles):
                w, s, r = m.tile_win[t], m.tile_sec[t], m.tile_rel[t]
                idx = idxw[:16, t * 8:(t + 1) * 8].T.reshape(-1).astype(np.int64)
                Xg = feats[idx + s * cfg.HALF]
                S = (np.arange(128)[None, :] == dl[:, t:t + 1]) * wv[:, t:t + 1]
                aggT = Xg.T.astype(f32) @ S
                if d == 128:
                    meanT[r, :, w * WIN:(w + 1) * WIN] += aggT
                else:
                    meanT[2 * r, :, w * WIN:(w + 1) * WIN] += aggT[:128]
                    meanT[2 * r + 1, :, w * WIN:(w + 1) * WIN] += aggT[128:]
            outT = np.zeros((cfg.H, cfg.NPAD), f32)
            for k in range(K):
                outT += Wcats[li][k].T @ meanT[k]
            out = outT.T + biases[li].T.reshape(-1)[None, :]
            h = np.maximum(out, 0.0)
            mu = h.mean(-1, keepdims=True)
            var = ((h - mu) ** 2).mean(-1, keepdims=True)
            h = (h - mu) / np.sqrt(var + cfg.LN_EPS) * gs[li] + bs[li]
            out_all[c * cfg.NPAD:(c + 1) * cfg.NPAD] = h
        return out_all

    h1 = layer(0, tbl, cfg.DIN)
    h2 = layer(1, h1, cfg.H)
    h = h1 + h2

    if sim_gelu:
        gelu = lambda v: np.maximum(v, 0.0)
    else:
        gelu = lambda v: 0.5 * v * (1 + erf(v / np.sqrt(2.0)))
    w1 = np.array(m.mlp_w1_bf, f32)
    w2 = np.array(m.mlp_w2_bf, f32)
    w3 = np.array(m.mlp_w3_bf, f32)
    out = np.zeros((m.n_dec, 2), f32)
    for c in range(C):
        i0 = m.dec_idx0_w[c][:16].T.reshape(-1).astype(np.int64)
        i1 = m.dec_idx1_w[c][:16].T.reshape(-1).astype(np.int64)
        for (off, n, s0, s1) in m.dec_ops:
            z = np.concatenate([h[i0[off:off + n] + s0 * cfg.HALF],
                                h[i1[off:off + n] + s1 * cfg.HALF]], 1)
            a1 = gelu(z @ w1 + np.asarray(mlp_b1, f32))
            a2 = gelu(a1 @ w2 + np.asarray(mlp_b2, f32))
            o = a2 @ w3 + np.asarray(mlp_b3, f32)
            mp = m.dec_map[c, off:off + n]
            out[mp[mp >= 0]] = o[mp >= 0]
    return out


# ---------------------------------------------------------------- device program


def build_program(m: Meta, cfg: CFG, sim_gelu=False,
                  nogather=None):
    if nogather is None:
        nogather = os.environ.get("RGCN_NOGATHER", "0") == "1"
    stage = int(os.environ.get("RGCN_STAGE", "4"))
    from contextlib import ExitStack

    import concourse.bacc as bacc
    import concourse.tile as tile
    from concourse import mybir

    f32 = mybir.dt.float32
    bf16 = mybir.dt.bfloat16
    i16 = mybir.dt.int16
    Alu = mybir.AluOpType
    Act = mybir.ActivationFunctionType
    GELU = Act.Relu if sim_gelu else Act.Gelu

    C, WIN, H = cfg.NC, cfg.WIN, cfg.H
    T = m.ntiles
    NW = cfg.NW

    nc = bacc.Bacc("TRN2", target_bir_lowering=False, debug=False,
                   num_devices=C, name="rgcn")

    x_tbl = nc.dram_tensor("x_tbl", [cfg.TBL, cfg.DIN], bf16, kind="ExternalInput")
    eidx = nc.dram_tensor("eidx", [128, T * 8], i16, kind="ExternalInput")
    dlv = nc.dram_tensor("dlv", [128, T], f32, kind="ExternalInput")
    wv = nc.dram_tensor("wv", [128, T], f32, kind="ExternalInput")
    wcat1 = nc.dram_tensor("wcat1", [m.K1 * 128, H], bf16, kind="ExternalInput")
    wcat2 = nc.dram_tensor("wcat2", [m.K2 * 128, H], bf16, kind="ExternalInput")
    b1h = nc.dram_tensor("b1h", [128, 2], f32, kind="ExternalInput")
    b2h = nc.dram_tensor("b2h", [128, 2], f32, kind="ExternalInput")
    g1bc = nc.dram_tensor("g1bc", [128, H], f32, kind="ExternalInput")
    b1lnbc = nc.dram_tensor("b1lnbc", [128, H], f32, kind="ExternalInput")
    g2bc = nc.dram_tensor("g2bc", [128, H], f32, kind="ExternalInput")
    b2lnbc = nc.dram_tensor("b2lnbc", [128, H], f32, kind="ExternalInput")
    mw1 = nc.dram_tensor("mw1", [512, 256], bf16, kind="ExternalInput")
    mw2 = nc.dram_tensor("mw2", [256, 128], bf16, kind="ExternalInput")
    mw3 = nc.dram_tensor("mw3", [128, 2], bf16, kind="ExternalInput")
    mb1 = nc.dram_tensor("mb1", [128, 2], f32, kind="ExternalInput")
    mb2 = nc.dram_tensor("mb2", [128, 1], f32, kind="ExternalInput")
    mb3 = nc.dram_tensor("mb3", [2, 1], f32, kind="ExternalInput")
    iota_in = nc.dram_tensor("iota", [128, 128], bf16, kind="ExternalInput")
    ident_in = nc.dram_tensor("ident", [128, 128], f32, kind="ExternalInput")
    didx0 = nc.dram_tensor("didx0", [128, m.DEC_TOT // 16], i16, kind="ExternalInput")
    didx1 = nc.dram_tensor("didx1", [128, m.DEC_TOT // 16], i16, kind="ExternalInput")

    y = nc.dram_tensor("y", [2, m.DEC_TOT], f32, kind="ExternalOutput")

    with tile.TileContext(nc) as tc, ExitStack() as es:
        dram = es.enter_context(tc.tile_pool(name="dram", bufs=1, space="DRAM"))
        const = es.enter_context(tc.tile_pool(name="const", bufs=1))
        resid = es.enter_context(tc.tile_pool(name="resid", bufs=1))

        _cnames = [0]
        def load_const(src, shape, dtype):
            _cnames[0] += 1
            cname = f"const{_cnames[0]}"
            t = const.tile(shape, dtype, tag=cname, name=cname)
            nc.sync.dma_start(t[:], src)
            return t

        iota_t = load_const(iota_in[:], [128, 128], bf16)
        ident_t = load_const(ident_in[:], [128, 128], f32)
        eidx_t = load_const(eidx[:], [128, T * 8], i16)
        dl_t = load_const(dlv[:], [128, T], f32)
        wv_t = load_const(wv[:], [128, T], f32)
        wc1_t = load_const(wcat1[:].rearrange("(k p) h -> p k h", p=128),
                           [128, m.K1, H], bf16)
        wc2_t = load_const(wcat2[:].rearrange("(k p) h -> p k h", p=128),
                           [128, m.K2, H], bf16)
        b1h_t = load_const(b1h[:], [128, 2], f32)
        b2h_t = load_const(b2h[:], [128, 2], f32)
        g1_t = load_const(g1bc[:], [128, H], f32)
        b1ln_t = load_const(b1lnbc[:], [128, H], f32)
        g2_t = load_const(g2bc[:], [128, H], f32)
        b2ln_t = load_const(b2lnbc[:], [128, H], f32)
        mw1_t = load_const(mw1[:].rearrange("(k p) h -> p k h", p=128),
                           [128, 4, 256], bf16)
        mw2_t = load_const(mw2[:].rearrange("(k p) h -> p k h", p=128),
                           [128, 2, 128], bf16)
        mw3_t = load_const(mw3[:], [128, 2], bf16)
        mb1_t = load_const(mb1[:], [128, 2], f32)
        mb2_t = load_const(mb2[:], [128, 1], f32)
        mb3_t = load_const(mb3[:], [2, 1], f32)
        didx0_t = load_const(didx0[:], [128, m.DEC_TOT // 16], i16)
        didx1_t = load_const(didx1[:], [128, m.DEC_TOT // 16], i16)

        h1res = resid.tile([128, NW, H], bf16, tag="h1res")
        eps_t = const.tile([128, 1], f32, tag="eps_t", name="eps_t")
        nc.vector.memset(eps_t[:], float(cfg.LN_EPS))

        h1slice = dram.tile([cfg.NPAD, H], bf16)
        hslice = dram.tile([cfg.NPAD, H], bf16)
        h1full = dram.tile([cfg.TBL, H], bf16, addr_space="Shared")
        hfull = dram.tile([cfg.TBL, H], bf16, addr_space="Shared")

        # precompute per-window tile lists
        win_tiles = [[] for _ in range(NW)]
        for t in range(T):
            win_tiles[m.tile_win[t]].append(t)
        win_gops = [[] for _ in range(NW)]
        for op in m.gather_ops:
            win_gops[m.tile_win[op[0]]].append(op)

        def emit_layer(li, lo_ap, hi_ap, full_ap, d_in, K, wc_t, bh_t,
                       g_t, bln_t, out_slice, add_residual):
            with ExitStack() as ls:
                gpool = ls.enter_context(tc.tile_pool(name=f"g{li}", bufs=3))
                spool = ls.enter_context(tc.tile_pool(name=f"s{li}", bufs=4))
                nbanks_ = (K + 3) // 4
                scatps = ls.enter_context(
                    tc.tile_pool(name=f"scat{li}", bufs=nbanks_ + (1 if nbanks_ < 5 else 0),
                                 space="PSUM"))
                meanp = ls.enter_context(tc.tile_pool(name=f"mean{li}", bufs=2))
                tfps = ls.enter_context(tc.tile_pool(name=f"tf{li}", bufs=1, space="PSUM"))
                trps = ls.enter_context(tc.tile_pool(name=f"tr{li}", bufs=1, space="PSUM"))
                lnp = ls.enter_context(tc.tile_pool(name=f"ln{li}", bufs=2))
                nbanks = (K + 3) // 4
                TG = cfg.TGROUP
                for g0 in range(0, NW, TG):
                    gwins = list(range(g0, min(g0 + TG, NW)))
                    gn = len(gwins)
                    meanT = meanp.tile([128, K, TG * WIN], bf16, tag="meanT")
                    for w in gwins:
                        wi = w - g0
                        banks = [scatps.tile([128, 512], f32, tag="scat", name="scatbank")
                                 for _ in range(nbanks)]
                        xg_tiles = {}
                        for (t0, nt, s) in win_gops[w]:
                            xg = gpool.tile([128, cfg.MAXB, d_in], bf16, tag="xg")
                            src_ap = lo_ap if s == 0 else hi_ap
                            if nogather:
                                nc.sync.dma_start(
                                    xg[:, :nt, :],
                                    full_ap[0:nt * 128, :].rearrange(
                                        "(t p) d -> p t d", p=128))
                            else:
                                nc.gpsimd.dma_gather(
                                    xg[:, :nt, :], src_ap,
                                    eidx_t[:, t0 * 8:(t0 + nt) * 8],
                                    nt * 128, nt * 128, d_in, elem_step=d_in,
                                    single_packet=False,
                                )
                            for j in range(nt):
                                xg_tiles[t0 + j] = (xg, j)
                        bank_touch = [[] for _ in range(nbanks)]
                        for t in win_tiles[w]:
                            r = m.tile_rel[t]
                            chunks = [r] if d_in == 128 else [2 * r, 2 * r + 1]
                            for ci, k in enumerate(chunks):
                                bank_touch[k // 4].append((t, ci, k))
                        first_t = {tk[:2]: (i == 0) for b in range(nbanks)
                                   for i, tk in enumerate(bank_touch[b]) for _ in [0]}
                        flags = {}
                        for b in range(nbanks):
                            for i, (t, ci, k) in enumerate(bank_touch[b]):
                                flags[(t, ci)] = (i == 0, i == len(bank_touch[b]) - 1)
                        for t in win_tiles[w]:
                            r = m.tile_rel[t]
                            xg, j = xg_tiles[t]
                            s_tile = spool.tile([128, 128], bf16, tag="stile")
                            nc.vector.tensor_scalar(
                                s_tile[:], iota_t[:],
                                dl_t[:, t:t + 1], wv_t[:, t:t + 1],
                                Alu.is_equal, Alu.mult,
                            )
                            chunks = [r] if d_in == 128 else [2 * r, 2 * r + 1]
                            for ci, k in enumerate(chunks):
                                st, sp = flags[(t, ci)]
                                nc.tensor.matmul(
                                    banks[k // 4][:, (k % 4) * 128:(k % 4) * 128 + 128],
                                    xg[:, j, ci * 128:(ci + 1) * 128],
                                    s_tile[:],
                                    start=st, stop=sp,
                                )
                        for bi in range(nbanks):
                            kk = min(4, K - bi * 4)
                            nc.scalar.copy(
                                meanT[:, bi * 4:bi * 4 + kk, wi * WIN:(wi + 1) * WIN],
                                banks[bi][:, 0:kk * 128].rearrange(
                                    "p (k n) -> p k n", n=WIN),
                            )
                    t1T = []
                    for half in range(2):
                        ps = tfps.tile([128, TG * WIN], f32, tag=f"tf{half}")
                        for k in range(K):
                            nc.tensor.matmul(
                                ps[:, :gn * WIN],
                                wc_t[:, k, half * 128:(half + 1) * 128],
                                meanT[:, k, :gn * WIN],
                                start=(k == 0), stop=(k == K - 1),
                            )
                        tt = lnp.tile([128, TG * WIN], f32, tag=f"t1T{half}")
                        nc.scalar.activation(tt[:, :gn * WIN], ps[:, :gn * WIN],
                                             Act.Relu, bias=bh_t[:, half:half + 1])
                        t1T.append(tt)
                    for w in gwins:
                        wi = w - g0
                        ptr = trps.tile([128, 256], f32, tag="tr")
                        nc.tensor.matmul(ptr[:, 0:128],
                                         t1T[0][:, wi * WIN:(wi + 1) * WIN], ident_t[:],
                                         is_transpose=True, start=True, stop=False)
                        nc.tensor.matmul(ptr[:, 128:256],
                                         t1T[1][:, wi * WIN:(wi + 1) * WIN], ident_t[:],
                                         is_transpose=True, start=False, stop=True)
                        t1 = lnp.tile([128, H], f32, tag="t1")
                        sumv = lnp.tile([128, 1], f32, tag="sumv")
                        nc.scalar.activation(t1[:], ptr[:], Act.Copy, accum_out=sumv[:])
                        mu = lnp.tile([128, 1], f32, tag="mu")
                        nc.scalar.mul(mu[:], sumv[:], 1.0 / H)
                        cent = lnp.tile([128, H], f32, tag="cent")
                        nc.vector.tensor_scalar(cent[:], t1[:], mu[:], None, Alu.subtract)
                        sq = lnp.tile([128, H], f32, tag="sq")
                        nc.vector.tensor_tensor(sq[:], cent[:], cent[:], Alu.mult)
                        varps = lnp.tile([128, 1], f32, tag="varps")
                        nc.vector.tensor_reduce(varps[:], sq[:],
                                                mybir.AxisListType.X, Alu.add)
                        stdv = lnp.tile([128, 1], f32, tag="stdv")
                        nc.scalar.activation(stdv[:], varps[:], Act.Sqrt,
                                             bias=eps_t[:], scale=1.0 / H)
                        rstd = lnp.tile([128, 1], f32, tag="rstd")
                        nc.vector.reciprocal(rstd[:], stdv[:])
                        normed = lnp.tile([128, H], f32, tag="normed")
                        nc.vector.tensor_scalar(normed[:], cent[:], rstd[:], None, Alu.mult)
                        tmpg = lnp.tile([128, H], f32, tag="tmpg")
                        nc.vector.tensor_tensor(tmpg[:], normed[:], g_t[:], Alu.mult)
                        if add_residual:
                            hw_f = lnp.tile([128, H], f32, tag="hw_f")
                            nc.vector.tensor_tensor(hw_f[:], tmpg[:], bln_t[:], Alu.add)
                            hw = lnp.tile([128, H], bf16, tag="hw")
                            nc.vector.tensor_tensor(hw[:], hw_f[:], h1res[:, w, :], Alu.add)
                            nc.sync.dma_start(out_slice[w * WIN:(w + 1) * WIN, :], hw[:])
                        else:
                            nc.vector.tensor_tensor(h1res[:, w, :], tmpg[:], bln_t[:],
                                                    Alu.add)
                            nc.sync.dma_start(out_slice[w * WIN:(w + 1) * WIN, :],
                                              h1res[:, w, :])

        emit_layer(1, x_tbl[0:cfg.HALF, :], x_tbl[cfg.HALF:, :], x_tbl[:], cfg.DIN,
                   m.K1, wc1_t, b1h_t, g1_t, b1ln_t, h1slice, False)
        if stage >= 2:
            nc.gpsimd.collective_compute(
                "AllGather", Alu.bypass, replica_groups=[list(range(C))],
                ins=[h1slice.opt()], outs=[h1full.opt()],
            )
        if stage >= 3:
            emit_layer(2, h1full[0:cfg.HALF, :], h1full[cfg.HALF:, :], h1full[:], H,
                       m.K2, wc2_t, b2h_t, g2_t, b2ln_t, hslice, True)
            nc.gpsimd.collective_compute(
                "AllGather", Alu.bypass, replica_groups=[list(range(C))],
                ins=[hslice.opt()], outs=[hfull.opt()],
            )
        if stage < 4:
            with tc.tile_pool(name="dummy", bufs=1) as dup:
                dy = dup.tile([2, m.DEC_TOT], f32, tag="dy", name="dy")
                nc.vector.memset(dy[:], 0.0)
                nc.sync.dma_start(y[:], dy[:])

        # ---------------- decoder
        if stage < 4:
            ds_skip = True
        with ExitStack() as ds:
            decp = None
            dec_ops_eff = m.dec_ops if stage >= 4 else []
            decp = ds.enter_context(tc.tile_pool(name="dec", bufs=3))
            decps = ds.enter_context(tc.tile_pool(name="decps", bufs=2, space="PSUM"))
            for (off, n, s0, s1) in dec_ops_eff:
                zt = decp.tile([128, 2, 2, n], bf16, tag="zt")
                for j, (sec, dt_) in enumerate([(s0, didx0_t), (s1, didx1_t)]):
                    src_ap = hfull[0:cfg.HALF, :] if sec == 0 else hfull[cfg.HALF:, :]
                    if nogather:
                        nc.vector.memset(zt[:, j, :, :], 0.0)
                    else:
                        nc.gpsimd.dma_gather(
                            zt[:, j, :, :], src_ap,
                            dt_[:, off // 16:(off + n) // 16],
                            n, n, H, elem_step=H, transpose=True,
                            single_packet=False,
                        )
                for sub in range(0, n, 512):
                    sn = min(512, n - sub)
                    a1T = decp.tile([128, 2, 512], bf16, tag="a1T")
                    for halfm in range(2):
                        ps = decps.tile([128, 512], f32, tag="dps")
                        for k in range(4):
                            nc.tensor.matmul(
                                ps[:, :sn],
                                mw1_t[:, k, halfm * 128:(halfm + 1) * 128],
                                zt[:, k // 2, k % 2, sub:sub + sn],
                                start=(k == 0), stop=(k == 3),
                            )
                        nc.scalar.activation(a1T[:, halfm, :sn], ps[:, :sn], GELU,
                                             bias=mb1_t[:, halfm:halfm + 1])
                    ps2 = decps.tile([128, 512], f32, tag="dps2")
                    for k in range(2):
                        nc.tensor.matmul(ps2[:, :sn], mw2_t[:, k, :], a1T[:, k, :sn],
                                         start=(k == 0), stop=(k == 1))
                    a2T = decp.tile([128, 512], bf16, tag="a2T")
                    nc.scalar.activation(a2T[:, :sn], ps2[:, :sn], GELU,
                                         bias=mb2_t[:])
                    ps3 = decps.tile([2, 512], f32, tag="dps3")
                    nc.tensor.matmul(ps3[:, :sn], mw3_t[:], a2T[:, :sn],
                                     start=True, stop=True)
                    stage = decp.tile([2, 512], f32, tag="stage")
                    nc.scalar.activation(stage[:, :sn], ps3[:, :sn],
                                         Act.Identity, bias=mb3_t[:])
                    nc.sync.dma_start(y[:, off + sub:off + sub + sn], stage[:, :sn])

    nc.finalize()
    return nc


# ---------------------------------------------------------------- run plumbing


def make_in_maps(m: Meta, cfg: CFG):
    maps = []
    for c in range(cfg.NC):
        maps.append({
            "x_tbl": np.asarray(m.x_tbl_bf),
            "eidx": m.idx_wrapped[c],
            "dlv": np.asarray(m.dl_bf[c]),
            "wv": np.asarray(m.w_bf[c]),
            "wcat1": np.asarray(m.Wcat1_bf).reshape(m.K1 * 128, cfg.H),
            "wcat2": np.asarray(m.Wcat2_bf).reshape(m.K2 * 128, cfg.H),
            "b1h": m.b1_half, "b2h": m.b2_half,
            "g1bc": m.g1_bc, "b1lnbc": m.b1ln_bc,
            "g2bc": m.g2_bc, "b2lnbc": m.b2ln_bc,
            "mw1": np.asarray(m.mlp_w1_bf), "mw2": np.asarray(m.mlp_w2_bf),
            "mw3": np.asarray(m.mlp_w3_bf),
            "mb1": m.mlp_b1_half, "mb2": m.mlp_b2, "mb3": m.mlp_b3,
            "iota": np.asarray(m.iota_bf), "ident": m.ident_f32,
            "didx0": m.dec_idx0_w[c], "didx1": m.dec_idx1_w[c],
        })
    return maps


def assemble_output(m: Meta, cfg: CFG, results):
    out = np.zeros((m.n_dec, 2), np.float32)
    for c in range(cfg.NC):
        yc = np.asarray(results[c]["y"])
        valid = m.dec_map[c] >= 0
        out[m.dec_map[c][valid]] = yc[:, valid].T
    return out


def run_sim(nc, m, cfg, in_maps):
    from concourse.bass_interp import MultiCoreSim
    sim = MultiCoreSim(nc, num_cores=cfg.NC, num_workers=int(os.environ.get("RGCN_SIM_WORKERS", "8")),
                       trace=False, require_finite=False, require_nnan=True)
    sims = [sim.cores[c] for c in range(cfg.NC)]
    for c in range(cfg.NC):
        for name, arr in in_maps[c].items():
            sims[c].tensor(name)[:] = arr
    sim.simulate(check_with_hw=False)
    return [{"y": np.array(sims[c].tensor("y"))} for c in range(cfg.NC)]


def kernel(**inputs):
    cfg = CFG(int(inputs["x"].shape[0]))
    m = preprocess(cfg=cfg, **inputs)
    simmode = os.environ.get("RGCN_SIM", "0") == "1"
    ck = ("prog", simmode, os.environ.get("RGCN_NOGATHER", "0"),
          os.environ.get("RGCN_STAGE", "4"))
    if ck not in _CACHE:
        _CACHE[ck] = build_program(m, cfg, sim_gelu=simmode)
    nc = _CACHE[ck]
    in_maps = make_in_maps(m, cfg)
    if os.environ.get("RGCN_SIM", "0") == "1":
        results = run_sim(nc, m, cfg, in_maps)
        return assemble_output(m, cfg, results)
    from concourse import bass_utils
    trace = bool(int(os.environ.get("RGCN_TRACE", "0")))
    res = bass_utils.run_bass_kernel_spmd(nc, in_maps, core_ids=list(range(cfg.NC)),
                                          trace=trace)
    if trace and res.exec_time_ns is not None:
        print(f"HW exec time: {res.exec_time_ns} ns")
    kernel.last_results = res
    return assemble_output(m, cfg, res.results)


_CACHE = {}
